# revision 49
# baseline (speedup 1.0000x reference)
"""Multi-head self-attention (batch=2, seq=2048, embed=1024, heads=16, causal)
sharded over 8 NeuronCores: data-parallel over batch (x2) and tensor-parallel
over heads (x4 groups of 4 heads).

Each core computes, for its (batch b, head group g):
  qkvT proj (transposed activations), causal softmax attention with the
  denominator folded into the AV matmul via a ones-column on V, and a partial
  output projection W_out[:, cols_g].T @ o_hat in transposed layout.
Host sums the 4 partials per batch, transposes back, and adds the constant
row  b_out + W_out @ b_v  (exact bias algebra).

Perf structure: all inputs are pre-tiled host-side so every DMA is a single
contiguous [128, N] transfer; diagonal k-tiles are trimmed to the causal
region (matmul N, exp N, and the mask mul all shrink); the K=64 logits
matmuls row-tile two heads concurrently on the PE array (auto tile_position
from base_partition).
"""

import os

import ml_dtypes
import numpy as np
from contextlib import ExitStack

import concourse.bass as bass
import concourse.mybir as mybir
import concourse.tile as tile
from concourse import bacc
from concourse.bass_utils import run_bass_kernel_spmd

N_HEADS = 16
EMBED = 1024
HEAD = 64
SEQ = 2048
BATCH = 2
N_CORES = 8
HPC = 4                # heads per core
GCOLS = HPC * HEAD     # 256 embed columns per head group
P = 128
CH = 512               # seq chunk
NCH = SEQ // CH        # 4
KT = SEQ // P          # 16 k tiles
VW = HPC * (HEAD + 1)  # v tile width per ktile (ones column at HEAD)

DT = mybir.dt.float32
DTB = mybir.dt.bfloat16
DTH = mybir.dt.float16

LAST_EXEC_NS = None
LAST_RESULTS = None


def _build_program():
    nc = bacc.Bacc("TRN2", target_bir_lowering=False, debug=False,
                   num_devices=N_CORES)
    # pre-tiled host layouts: every tensor is [128, ...] with the exact SBUF
    # column order so each load is one contiguous DMA
    xT = nc.dram_tensor("xT", [P, NCH * 8 * CH], DTB, kind="ExternalInput")
    wqkT = nc.dram_tensor("wqkT", [P, 8 * 2 * GCOLS], DTB, kind="ExternalInput")
    wvT = nc.dram_tensor("wvT", [P, 8 * GCOLS], DTB, kind="ExternalInput")
    bqk = nc.dram_tensor("bqk", [P, 4], DT, kind="ExternalInput")
    woT = nc.dram_tensor("woT", [P, 2 * EMBED], DTB, kind="ExternalInput")
    mask2 = nc.dram_tensor("mask2", [P, 2 * P], DTB, kind="ExternalInput")
    yT = nc.dram_tensor("yT", [P, NCH * 8 * CH], DTH, kind="ExternalOutput")

    with tile.TileContext(nc) as tc, ExitStack() as ctx:
        const = ctx.enter_context(tc.tile_pool(name="const", bufs=1))
        xpool = ctx.enter_context(tc.tile_pool(name="xpool", bufs=4))
        stpool = ctx.enter_context(tc.tile_pool(name="stpool", bufs=10))
        small = ctx.enter_context(tc.tile_pool(name="small", bufs=6))
        outsb = ctx.enter_context(tc.tile_pool(name="outsb", bufs=3))
        ps512 = ctx.enter_context(tc.tile_pool(name="ps512", bufs=2, space="PSUM"))
        stP = ctx.enter_context(tc.tile_pool(name="stP", bufs=2, space="PSUM"))
        psO = ctx.enter_context(tc.tile_pool(name="psO", bufs=2, space="PSUM"))

        # ---- persistent SBUF residents ----
        wqkbig = const.tile([P, 8 * 2 * GCOLS], DTB, tag="wqkbig")
        wvbig = const.tile([P, 8 * GCOLS], DTB, tag="wvbig")
        wobig = const.tile([P, 2 * EMBED], DTB, tag="wobig")
        bqk_sb = const.tile([P, 4], DT, tag="bqk")
        mask_sb = const.tile([P, 2 * P], DTB, tag="mask")
        # per-chunk tiles: the Tile framework tracks deps at tile granularity,
        # so chunked q/k/v/ohat avoid false write-read serialization between
        # qkv of a later chunk and attention/outproj of an earlier one
        qt_c = [[const.tile([P, CH], DTB, tag=f"qt{a}c{q}", name=f"qt{a}c{q}")
                 for q in range(NCH)] for a in range(2)]
        kt_c = [[const.tile([P, CH], DTB, tag=f"kt{a}c{q}", name=f"kt{a}c{q}")
                 for q in range(NCH)] for a in range(2)]
        vt_c = [const.tile([P, 4 * VW], DTB, tag=f"vtc{q}", name=f"vtc{q}")
                for q in range(NCH)]
        ohat_c = [[const.tile([P, CH], DTB, tag=f"oh{a}c{q}", name=f"oh{a}c{q}")
                   for q in range(NCH)] for a in range(2)]

        def load_x(qi, eng):
            xb = xpool.tile([P, 8 * CH], DTB, tag="xt", name=f"x{qi}")
            eng.dma_start(out=xb, in_=xT[:, bass.ds(8 * CH * qi, 8 * CH)])
            return xb

        # initial loads. wqk and x0 are pieced (256KB) and alternated across
        # the two HWDGE rings (SDMA round-robins rings at packet granularity,
        # so both rings must carry critical bytes); everything non-critical
        # queues strictly behind them to keep full HBM bandwidth on the
        # chunk-0 critical path.
        x0 = xpool.tile([P, 8 * CH], DTB, tag="xt", name="x0")
        # piece widths: a tiny first piece (i=0 only) lets the first matmul
        # start ~2us earlier; later pieces are bigger to amortize issue cost
        pieces = [(0, CH), (CH, 3 * CH), (4 * CH, 2 * CH), (6 * CH, 2 * CH)]
        for pc, (o, w) in enumerate(pieces):
            sl = bass.ds(o, w)
            ew, ex = (nc.sync, nc.scalar) if pc % 2 == 0 else (nc.scalar, nc.sync)
            ew.dma_start(out=wqkbig[:, sl], in_=wqkT[:, sl])
            ex.dma_start(out=x0[:, sl], in_=xT[:, sl])
        nc.gpsimd.dma_start(out=bqk_sb, in_=bqk[:])
        nc.gpsimd.dma_start(out=mask_sb, in_=mask2[:])
        nc.sync.dma_start(out=wvbig, in_=wvT[:])
        x1 = load_x(1, nc.sync)
        x2 = load_x(2, nc.scalar)
        x3 = load_x(3, nc.sync)
        nc.scalar.dma_start(out=wobig, in_=woT[:])
        # HAM warmup: ~4us of dummy matmuls on a zeroed tile during the
        # initial DMA wait, so the PE clock gate is already at 2.4GHz when
        # the first real (DMA-paced) matmuls arrive
        wusrc = const.tile([P, CH], DTB, tag="wusrc")
        nc.vector.memset(wusrc[:], 0.0)
        wups = ps512.tile([P, CH], DT, tag="ps512", name="wups")
        for _ in range(10):
            nc.tensor.matmul(wups, lhsT=wusrc[:, 0:P], rhs=wusrc[:],
                             start=True, stop=True)
        # ones column per (ktile, head) at offset HEAD in each v block
        for q in range(NCH):
            nc.vector.memset(
                vt_c[q].rearrange("p (t h d) -> p t h d",
                                  t=4, h=HPC)[:, :, :, HEAD:HEAD + 1],
                1.0)


        def qkv_chunk(qi, xb):
            sl = bass.ds(CH * qi, CH)
            if qi == 0:
                # i-major with 4 accumulators (attn pools are idle during
                # chunk 0, so f2/f3 borrow stP) — all 4 f matmuls consume each
                # x/wqk piece as it lands, keeping PE at DMA arrival rate
                pss = [ps512.tile([P, CH], DT, tag="ps512", name=f"psqk{f}")
                       for f in range(2)]
                pss += [stP.tile([P, CH], DT, tag="stP", name=f"psqk{f}")
                        for f in range(2, 4)]
                for i in range(8):
                    for f in range(4):
                        nc.tensor.matmul(
                            pss[f],
                            lhsT=wqkbig[:, bass.ds(2 * GCOLS * i + P * f, P)],
                            rhs=xb[:, bass.ds(CH * i, CH)],
                            start=(i == 0), stop=(i == 7),
                        )
                for f in range(4):
                    dst = qt_c[f][qi] if f < 2 else kt_c[f - 2][qi]
                    nc.vector.tensor_scalar_add(dst[:], pss[f],
                                                bqk_sb[:, f:f + 1])
            else:
                for f in range(4):
                    ps = ps512.tile([P, CH], DT, tag="ps512", name="psqk")
                    for i in range(8):
                        nc.tensor.matmul(
                            ps,
                            lhsT=wqkbig[:, bass.ds(2 * GCOLS * i + P * f, P)],
                            rhs=xb[:, bass.ds(CH * i, CH)],
                            start=(i == 0), stop=(i == 7),
                        )
                    dst = qt_c[f][qi] if f < 2 else kt_c[f - 2][qi]
                    nc.vector.tensor_scalar_add(dst[:], ps, bqk_sb[:, f:f + 1])
            for s in range(4):
                ps = ps512.tile([P, GCOLS], DT, tag="ps512", name="psv")
                for i in range(8):
                    nc.tensor.matmul(
                        ps,
                        lhsT=xb[:, bass.ds(CH * i + P * s, P)],
                        rhs=wvbig[:, bass.ds(GCOLS * i, GCOLS)],
                        start=(i == 0), stop=(i == 7),
                    )
                dst = vt_c[qi][:, bass.ds(VW * s, VW)].rearrange(
                    "p (h d) -> p h d", h=HPC)[:, :, 0:HEAD]
                nc.vector.tensor_copy(dst, ps.rearrange("p (h d) -> p h d", h=HPC))

        def attn_pair(hp, qi):
            nk = 4 * qi + 4
            po = [psO.tile([HEAD + 1, CH], DT, tag="psO", name="psO")
                  for _ in range(2)]
            for ki in range(nk):
                kr = ki - 4 * qi
                off = P * kr if kr > 0 else 0   # causal trim for diagonal tiles
                W = CH - off
                pst = stP.tile([P, 2 * CH], DT, tag="stP", name="pst")
                for hh in range(2):
                    r0 = HEAD * hh
                    nc.tensor.matmul(
                        pst[:, bass.ds(CH * hh + off, W)],
                        lhsT=kt_c[hp][ki // 4][r0:r0 + HEAD, bass.ds(P * (ki % 4), P)],
                        rhs=qt_c[hp][qi][r0:r0 + HEAD, bass.ds(off, W)],
                        start=True, stop=True,
                    )
                st = stpool.tile([P, 2 * CH], DTB, tag="st", name="st")
                stv = st[:, 0:2 * W].rearrange("p (h w) -> p h w", h=2)
                nc.scalar.activation(
                    stv,
                    pst.rearrange("p (h c) -> p h c", h=2)[:, :, off:CH],
                    mybir.ActivationFunctionType.Exp,
                    scale=0.125)
                if kr >= 0:
                    nc.vector.tensor_mul(
                        stv[:, :, 0:P], stv[:, :, 0:P],
                        mask_sb.rearrange("p (h c) -> p h c", h=2))
                for hh in range(2):
                    h = 2 * hp + hh
                    nc.tensor.matmul(
                        po[hh][:, bass.ds(off, W)],
                        lhsT=vt_c[ki // 4][:, bass.ds(VW * (ki % 4) + (HEAD + 1) * h,
                                                      HEAD + 1)],
                        rhs=st[:, bass.ds(W * hh, W)],
                        start=(ki == 0), stop=(ki == nk - 1),
                    )
            # per-hh pipelined den chain: copy/recip/broadcast/mul stages of
            # the two heads overlap across the vector and gpsimd queues
            last_pair = (hp == 1 and qi == NCH - 1)
            den = small.tile([1, 2 * CH], DT, tag="den", name="den")
            nc.vector.tensor_copy(den[:, 0:CH], po[0][HEAD:HEAD + 1, :])
            # scalar is exp-free at the very end: parallelize the tail chain
            (nc.scalar.copy if last_pair else nc.vector.tensor_copy)(
                den[:, bass.ds(CH, CH)], po[1][HEAD:HEAD + 1, :])
            recip = small.tile([1, 2 * CH], DT, tag="recip", name="recip")
            recipb = small.tile([HEAD, 2 * CH], DT, tag="recipb", name="recipb")
            for hh in range(2):
                nc.vector.reciprocal_approx_fast(
                    recip[:, bass.ds(CH * hh, CH)], den[:, bass.ds(CH * hh, CH)])
                nc.gpsimd.partition_broadcast(
                    recipb[:, bass.ds(CH * hh, CH)], recip[:, bass.ds(CH * hh, CH)])
            for hh in range(2):
                nc.vector.tensor_mul(
                    ohat_c[hp][qi][HEAD * hh:HEAD * hh + HEAD, :],
                    po[hh][0:HEAD, :], recipb[:, bass.ds(CH * hh, CH)])

        def outproj_chunk(qi, pool=None):
            last = pool is not None
            obig = outsb.tile([P, 8 * CH], DTH, tag="ot", name="obig")

            def cast_and_dma(m, ps, gsz):
                if last:
                    # scalar engine is exp-free by the last chunk
                    nc.scalar.copy(obig[:, bass.ds(CH * m, CH // 2)],
                                   ps[:, 0:CH // 2])
                    nc.vector.tensor_copy(
                        obig[:, bass.ds(CH * m + CH // 2, CH // 2)],
                        ps[:, bass.ds(CH // 2, CH // 2)])
                else:
                    nc.vector.tensor_copy(obig[:, bass.ds(CH * m, CH)], ps)
                if m % gsz == gsz - 1:
                    grp = m // gsz
                    deng = nc.scalar if (last and grp % 2 == 1) else nc.sync
                    deng.dma_start(
                        out=yT[:, bass.ds(8 * CH * qi + gsz * CH * grp, gsz * CH)],
                        in_=obig[:, bass.ds(gsz * CH * grp, gsz * CH)])

            if not last:
                for m in range(8):
                    ps = ps512.tile([P, CH], DT, tag="ps512", name="pso")
                    for k in range(2):
                        nc.tensor.matmul(
                            ps,
                            lhsT=wobig[:, bass.ds(EMBED * k + P * m, P)],
                            rhs=ohat_c[k][qi][:],
                            start=(k == 0), stop=(k == 1),
                        )
                    cast_and_dma(m, ps, 4)
                return
            # last chunk: two-phase — the k=0 matmuls only need the hp0 ohat
            # (ready long before) and run during the final den chain, keeping
            # the PE warm; k=1 lands as soon as the hp1 normalize finishes
            t01 = stP.tile([P, 2 * CH], DT, tag="stP", name="op3a")
            t23 = stP.tile([P, 2 * CH], DT, tag="stP", name="op3b")
            accs = [t01[:, 0:CH], t01[:, bass.ds(CH, CH)],
                    t23[:, 0:CH], t23[:, bass.ds(CH, CH)]]
            accs += [ps512.tile([P, CH], DT, tag="ps512", name=f"op3c{j}")
                     for j in range(2)]
            # k=0 as closed groups so the scheduler runs them during the den
            # chain instead of pairing each with its (blocked) k=1 partner;
            # k=1 then accumulates onto the surviving has_written state
            for m in range(6):
                nc.tensor.matmul(
                    accs[m], lhsT=wobig[:, bass.ds(P * m, P)],
                    rhs=ohat_c[0][qi][:], start=True, stop=True)
            for m in range(6):
                nc.tensor.matmul(
                    accs[m], lhsT=wobig[:, bass.ds(EMBED + P * m, P)],
                    rhs=ohat_c[1][qi][:], start=False, stop=True,
                    skip_group_check=True)
                cast_and_dma(m, accs[m], 2)
            for m in (6, 7):
                ps = stP.tile([P, CH], DT, tag="stP", name="pso")
                for k in range(2):
                    nc.tensor.matmul(
                        ps,
                        lhsT=wobig[:, bass.ds(EMBED * k + P * m, P)],
                        rhs=ohat_c[k][qi][:],
                        start=(k == 0), stop=(k == 1),
                    )
                cast_and_dma(m, ps, 2)

        # software pipeline: fill PE stalls at pair boundaries with
        # independent QKV / out-proj work
        # schedule: every attn pair is exp-bound (ACT > its own PE work), so
        # the pure-PE fill work (qkv, outproj) is rationed across the windows:
        # qkv(qi+1) fills the qi window, and ALL outproj chunks are held back
        # for the heavily exp-bound qi=3 window.
        qkv_chunk(0, x0)
        attn_pair(0, 0)
        qkv_chunk(1, x1)
        attn_pair(1, 0)
        attn_pair(0, 1)
        qkv_chunk(2, x2)
        attn_pair(1, 1)
        attn_pair(0, 2)
        qkv_chunk(3, x3)
        attn_pair(1, 2)
        attn_pair(0, 3)
        outproj_chunk(0)
        attn_pair(1, 3)
        outproj_chunk(1)
        outproj_chunk(2)
        outproj_chunk(NCH - 1, pool=stP)

    nc.compile()
    return nc


def kernel(x, W_qkv, b_qkv, W_out, b_out):
    global LAST_EXEC_NS, LAST_RESULTS
    x = np.asarray(x, dtype=np.float32)
    W_qkv = np.asarray(W_qkv, dtype=np.float32)
    b_qkv = np.asarray(b_qkv, dtype=np.float32)
    W_out = np.asarray(W_out, dtype=np.float32)
    b_out = np.asarray(b_out, dtype=np.float32)

    nc = _build_program()

    tri = (np.arange(P)[:, None] <= np.arange(P)[None, :]).astype(np.float32)
    mask2 = np.concatenate([tri, tri], axis=1)          # [128, 256]

    in_maps = []
    for c in range(N_CORES):
        b, g = divmod(c, HPC)
        q0 = GCOLS * g
        wq = W_qkv[q0:q0 + GCOLS]                        # [256, 1024]
        wk = W_qkv[EMBED + q0:EMBED + q0 + GCOLS]
        wv = W_qkv[2 * EMBED + q0:2 * EMBED + q0 + GCOLS]
        bq = b_qkv[q0:q0 + GCOLS]
        bk = b_qkv[EMBED + q0:EMBED + q0 + GCOLS]
        bqk_h = np.stack([bq[0:P], bq[P:2 * P], bk[0:P], bk[P:2 * P]],
                         axis=1).astype(np.float32)      # [128, 4]
        xT = np.ascontiguousarray(x[b].T)                # [1024, 2048]
        xtile = xT.reshape(8, P, NCH, CH).transpose(1, 2, 0, 3).reshape(P, -1)
        wqkT = np.concatenate([wq, wk], 0).T             # [1024, 512]
        wqkt = wqkT.reshape(8, P, 2 * GCOLS).transpose(1, 0, 2).reshape(P, -1)
        wvt = wv.T.reshape(8, P, GCOLS).transpose(1, 0, 2).reshape(P, -1)
        wot = W_out[:, q0:q0 + GCOLS].T.reshape(2, P, EMBED).transpose(1, 0, 2).reshape(P, -1)
        in_maps.append({
            "xT": np.ascontiguousarray(xtile).astype(ml_dtypes.bfloat16),
            "wqkT": np.ascontiguousarray(wqkt).astype(ml_dtypes.bfloat16),
            "wvT": np.ascontiguousarray(wvt).astype(ml_dtypes.bfloat16),
            "bqk": np.ascontiguousarray(bqk_h),
            "woT": np.ascontiguousarray(wot).astype(ml_dtypes.bfloat16),
            "mask2": mask2.astype(ml_dtypes.bfloat16),
        })

    want_trace = bool(int(os.environ.get("KTRACE", "0")))
    if want_trace:
        try:
            import antenv.axon_hooks  # noqa: F401
        except ImportError:
            want_trace = False
    res = run_bass_kernel_spmd(nc, in_maps, list(range(N_CORES)),
                               trace=want_trace,
                               tmpdir=os.environ.get("KTRACE_DIR") or None)
    LAST_EXEC_NS = res.exec_time_ns
    LAST_RESULTS = res

    out = np.empty((BATCH, SEQ, EMBED), dtype=np.float32)
    crow = (b_out + W_out @ b_qkv[2 * EMBED:]).astype(np.float32)
    for b in range(BATCH):
        acc = np.zeros((EMBED, SEQ), dtype=np.float32)
        for g in range(HPC):
            yt = res.results[HPC * b + g]["yT"].astype(np.float32)
            acc += yt.reshape(P, NCH, 8, CH).transpose(2, 0, 1, 3).reshape(EMBED, SEQ)
        out[b] = acc.T + crow[None, :]
    return out


# revision 50
# speedup vs baseline: 1.1624x; 1.1624x over previous
"""Multi-head self-attention (batch=2, seq=2048, embed=1024, heads=16, causal)
sharded over 8 NeuronCores: data-parallel over batch (x2) and tensor-parallel
over heads (x4 groups of 4 heads).

Each core computes, for its (batch b, head group g):
  qkvT proj (transposed activations), causal softmax attention with the
  denominator folded into the AV matmul via a ones-column on V, and a partial
  output projection W_out[:, cols_g].T @ o_hat in transposed layout.
Host sums the 4 partials per batch, transposes back, and adds the constant
row  b_out + W_out @ b_v  (exact bias algebra).

Perf structure: all inputs are pre-tiled host-side so every DMA is a single
contiguous [128, N] transfer; diagonal k-tiles are trimmed to the causal
region (matmul N, exp N, and the mask mul all shrink); the K=64 logits
matmuls row-tile two heads concurrently on the PE array (auto tile_position
from base_partition).
"""

import os

import ml_dtypes
import numpy as np
from contextlib import ExitStack

import concourse.bass as bass
import concourse.mybir as mybir
import concourse.tile as tile
from concourse import bacc
from concourse.bass_utils import run_bass_kernel_spmd

N_HEADS = 16
EMBED = 1024
HEAD = 64
SEQ = 2048
BATCH = 2
N_CORES = 8
HPC = 4                # heads per core
GCOLS = HPC * HEAD     # 256 embed columns per head group
P = 128
CH = 512               # seq chunk
NCH = SEQ // CH        # 4
KT = SEQ // P          # 16 k tiles
VW = HPC * (HEAD + 1)  # v tile width per ktile (ones column at HEAD)

DT = mybir.dt.float32
DTB = mybir.dt.bfloat16
DTH = mybir.dt.float16

LAST_EXEC_NS = None
LAST_RESULTS = None


def _build_program():
    nc = bacc.Bacc("TRN2", target_bir_lowering=False, debug=False,
                   num_devices=N_CORES)
    # pre-tiled host layouts: every tensor is [128, ...] with the exact SBUF
    # column order so each load is one contiguous DMA
    xT = nc.dram_tensor("xT", [P, NCH * 8 * CH], DTB, kind="ExternalInput")
    wqkT = nc.dram_tensor("wqkT", [P, 8 * 2 * GCOLS], DTB, kind="ExternalInput")
    wvT = nc.dram_tensor("wvT", [P, 8 * GCOLS], DTB, kind="ExternalInput")
    bqk = nc.dram_tensor("bqk", [P, 4], DT, kind="ExternalInput")
    woT = nc.dram_tensor("woT", [P, 2 * EMBED], DTB, kind="ExternalInput")
    mask2 = nc.dram_tensor("mask2", [P, 2 * P], DTB, kind="ExternalInput")
    yT = nc.dram_tensor("yT", [P, NCH * 8 * CH], DTH, kind="ExternalOutput")

    with tile.TileContext(nc) as tc, ExitStack() as ctx:
        const = ctx.enter_context(tc.tile_pool(name="const", bufs=1))
        xpool = ctx.enter_context(tc.tile_pool(name="xpool", bufs=4))
        stpool = ctx.enter_context(tc.tile_pool(name="stpool", bufs=10))
        small = ctx.enter_context(tc.tile_pool(name="small", bufs=6))
        outsb = ctx.enter_context(tc.tile_pool(name="outsb", bufs=3))
        ps512 = ctx.enter_context(tc.tile_pool(name="ps512", bufs=2, space="PSUM"))
        stP = ctx.enter_context(tc.tile_pool(name="stP", bufs=2, space="PSUM"))
        psO = ctx.enter_context(tc.tile_pool(name="psO", bufs=2, space="PSUM"))

        # ---- persistent SBUF residents ----
        wqkbig = const.tile([P, 8 * 2 * GCOLS], DTB, tag="wqkbig")
        wvbig = const.tile([P, 8 * GCOLS], DTB, tag="wvbig")
        wobig = const.tile([P, 2 * EMBED], DTB, tag="wobig")
        bqk_sb = const.tile([P, 4], DT, tag="bqk")
        mask_sb = const.tile([P, 2 * P], DTB, tag="mask")
        # per-chunk tiles: the Tile framework tracks deps at tile granularity,
        # so chunked q/k/v/ohat avoid false write-read serialization between
        # qkv of a later chunk and attention/outproj of an earlier one
        qt_c = [[const.tile([P, CH], DTB, tag=f"qt{a}c{q}", name=f"qt{a}c{q}")
                 for q in range(NCH)] for a in range(2)]
        kt_c = [[const.tile([P, CH], DTB, tag=f"kt{a}c{q}", name=f"kt{a}c{q}")
                 for q in range(NCH)] for a in range(2)]
        vt_c = [const.tile([P, 4 * VW], DTB, tag=f"vtc{q}", name=f"vtc{q}")
                for q in range(NCH)]
        ohat_c = [[const.tile([P, CH], DTB, tag=f"oh{a}c{q}", name=f"oh{a}c{q}")
                   for q in range(NCH)] for a in range(2)]

        def load_x(qi, eng):
            xb = xpool.tile([P, 8 * CH], DTB, tag="xt", name=f"x{qi}")
            eng.dma_start(out=xb, in_=xT[:, bass.ds(8 * CH * qi, 8 * CH)])
            return xb

        # initial loads. wqk and x0 are pieced (256KB) and alternated across
        # the two HWDGE rings (SDMA round-robins rings at packet granularity,
        # so both rings must carry critical bytes); everything non-critical
        # queues strictly behind them to keep full HBM bandwidth on the
        # chunk-0 critical path.
        x0 = xpool.tile([P, 8 * CH], DTB, tag="xt", name="x0")
        # piece widths: a tiny first piece (i=0 only) lets the first matmul
        # start ~2us earlier; later pieces are bigger to amortize issue cost
        pieces = [(0, CH), (CH, 3 * CH), (4 * CH, 2 * CH), (6 * CH, 2 * CH)]
        for pc, (o, w) in enumerate(pieces):
            sl = bass.ds(o, w)
            ew, ex = (nc.sync, nc.scalar) if pc % 2 == 0 else (nc.scalar, nc.sync)
            ew.dma_start(out=wqkbig[:, sl], in_=wqkT[:, sl])
            ex.dma_start(out=x0[:, sl], in_=xT[:, sl])
        nc.gpsimd.dma_start(out=bqk_sb, in_=bqk[:])
        nc.gpsimd.dma_start(out=mask_sb, in_=mask2[:])
        nc.sync.dma_start(out=wvbig, in_=wvT[:])
        x1 = load_x(1, nc.sync)
        x2 = load_x(2, nc.scalar)
        x3 = load_x(3, nc.sync)
        nc.scalar.dma_start(out=wobig, in_=woT[:])
        # ones column per (ktile, head) at offset HEAD in each v block
        for q in range(NCH):
            nc.vector.memset(
                vt_c[q].rearrange("p (t h d) -> p t h d",
                                  t=4, h=HPC)[:, :, :, HEAD:HEAD + 1],
                1.0)


        def qkv_chunk(qi, xb):
            sl = bass.ds(CH * qi, CH)
            if qi == 0:
                # i-major with 4 accumulators (attn pools are idle during
                # chunk 0, so f2/f3 borrow stP) — all 4 f matmuls consume each
                # x/wqk piece as it lands, keeping PE at DMA arrival rate
                pss = [ps512.tile([P, CH], DT, tag="ps512", name=f"psqk{f}")
                       for f in range(2)]
                pss += [stP.tile([P, CH], DT, tag="stP", name=f"psqk{f}")
                        for f in range(2, 4)]
                for i in range(8):
                    for f in range(4):
                        nc.tensor.matmul(
                            pss[f],
                            lhsT=wqkbig[:, bass.ds(2 * GCOLS * i + P * f, P)],
                            rhs=xb[:, bass.ds(CH * i, CH)],
                            start=(i == 0), stop=(i == 7),
                        )
                for f in range(4):
                    dst = qt_c[f][qi] if f < 2 else kt_c[f - 2][qi]
                    nc.vector.tensor_scalar_add(dst[:], pss[f],
                                                bqk_sb[:, f:f + 1])
            else:
                for f in range(4):
                    ps = ps512.tile([P, CH], DT, tag="ps512", name="psqk")
                    for i in range(8):
                        nc.tensor.matmul(
                            ps,
                            lhsT=wqkbig[:, bass.ds(2 * GCOLS * i + P * f, P)],
                            rhs=xb[:, bass.ds(CH * i, CH)],
                            start=(i == 0), stop=(i == 7),
                        )
                    dst = qt_c[f][qi] if f < 2 else kt_c[f - 2][qi]
                    nc.vector.tensor_scalar_add(dst[:], ps, bqk_sb[:, f:f + 1])
            for s in range(4):
                ps = ps512.tile([P, GCOLS], DT, tag="ps512", name="psv")
                for i in range(8):
                    nc.tensor.matmul(
                        ps,
                        lhsT=xb[:, bass.ds(CH * i + P * s, P)],
                        rhs=wvbig[:, bass.ds(GCOLS * i, GCOLS)],
                        start=(i == 0), stop=(i == 7),
                    )
                dst = vt_c[qi][:, bass.ds(VW * s, VW)].rearrange(
                    "p (h d) -> p h d", h=HPC)[:, :, 0:HEAD]
                nc.vector.tensor_copy(dst, ps.rearrange("p (h d) -> p h d", h=HPC))

        def attn_pair(hp, qi):
            nk = 4 * qi + 4
            po = [psO.tile([HEAD + 1, CH], DT, tag="psO", name="psO")
                  for _ in range(2)]
            for ki in range(nk):
                kr = ki - 4 * qi
                off = P * kr if kr > 0 else 0   # causal trim for diagonal tiles
                W = CH - off
                pst = stP.tile([P, 2 * CH], DT, tag="stP", name="pst")
                for hh in range(2):
                    r0 = HEAD * hh
                    nc.tensor.matmul(
                        pst[:, bass.ds(CH * hh + off, W)],
                        lhsT=kt_c[hp][ki // 4][r0:r0 + HEAD, bass.ds(P * (ki % 4), P)],
                        rhs=qt_c[hp][qi][r0:r0 + HEAD, bass.ds(off, W)],
                        start=True, stop=True,
                    )
                st = stpool.tile([P, 2 * CH], DTB, tag="st", name="st")
                stv = st[:, 0:2 * W].rearrange("p (h w) -> p h w", h=2)
                nc.scalar.activation(
                    stv,
                    pst.rearrange("p (h c) -> p h c", h=2)[:, :, off:CH],
                    mybir.ActivationFunctionType.Exp,
                    scale=0.125)
                if kr >= 0:
                    nc.vector.tensor_mul(
                        stv[:, :, 0:P], stv[:, :, 0:P],
                        mask_sb.rearrange("p (h c) -> p h c", h=2))
                for hh in range(2):
                    h = 2 * hp + hh
                    nc.tensor.matmul(
                        po[hh][:, bass.ds(off, W)],
                        lhsT=vt_c[ki // 4][:, bass.ds(VW * (ki % 4) + (HEAD + 1) * h,
                                                      HEAD + 1)],
                        rhs=st[:, bass.ds(W * hh, W)],
                        start=(ki == 0), stop=(ki == nk - 1),
                    )
            # per-hh pipelined den chain: copy/recip/broadcast/mul stages of
            # the two heads overlap across the vector and gpsimd queues
            last_pair = (hp == 1 and qi == NCH - 1)
            den = small.tile([1, 2 * CH], DT, tag="den", name="den")
            nc.vector.tensor_copy(den[:, 0:CH], po[0][HEAD:HEAD + 1, :])
            # scalar is exp-free at the very end: parallelize the tail chain
            (nc.scalar.copy if last_pair else nc.vector.tensor_copy)(
                den[:, bass.ds(CH, CH)], po[1][HEAD:HEAD + 1, :])
            recip = small.tile([1, 2 * CH], DT, tag="recip", name="recip")
            recipb = small.tile([HEAD, 2 * CH], DT, tag="recipb", name="recipb")
            for hh in range(2):
                nc.vector.reciprocal_approx_fast(
                    recip[:, bass.ds(CH * hh, CH)], den[:, bass.ds(CH * hh, CH)])
                nc.gpsimd.partition_broadcast(
                    recipb[:, bass.ds(CH * hh, CH)], recip[:, bass.ds(CH * hh, CH)])
            for hh in range(2):
                nc.vector.tensor_mul(
                    ohat_c[hp][qi][HEAD * hh:HEAD * hh + HEAD, :],
                    po[hh][0:HEAD, :], recipb[:, bass.ds(CH * hh, CH)])

        def outproj_chunk(qi, pool=None):
            last = pool is not None
            obig = outsb.tile([P, 8 * CH], DTH, tag="ot", name="obig")

            def cast_and_dma(m, ps, gsz):
                if last:
                    # scalar engine is exp-free by the last chunk
                    nc.scalar.copy(obig[:, bass.ds(CH * m, CH // 2)],
                                   ps[:, 0:CH // 2])
                    nc.vector.tensor_copy(
                        obig[:, bass.ds(CH * m + CH // 2, CH // 2)],
                        ps[:, bass.ds(CH // 2, CH // 2)])
                else:
                    nc.vector.tensor_copy(obig[:, bass.ds(CH * m, CH)], ps)
                if m % gsz == gsz - 1:
                    grp = m // gsz
                    deng = nc.scalar if (last and grp % 2 == 1) else nc.sync
                    deng.dma_start(
                        out=yT[:, bass.ds(8 * CH * qi + gsz * CH * grp, gsz * CH)],
                        in_=obig[:, bass.ds(gsz * CH * grp, gsz * CH)])

            if not last:
                for m in range(8):
                    ps = ps512.tile([P, CH], DT, tag="ps512", name="pso")
                    for k in range(2):
                        nc.tensor.matmul(
                            ps,
                            lhsT=wobig[:, bass.ds(EMBED * k + P * m, P)],
                            rhs=ohat_c[k][qi][:],
                            start=(k == 0), stop=(k == 1),
                        )
                    cast_and_dma(m, ps, 4)
                return
            # last chunk: two-phase — the k=0 matmuls only need the hp0 ohat
            # (ready long before) and run during the final den chain, keeping
            # the PE warm; k=1 lands as soon as the hp1 normalize finishes
            t01 = stP.tile([P, 2 * CH], DT, tag="stP", name="op3a")
            t23 = stP.tile([P, 2 * CH], DT, tag="stP", name="op3b")
            accs = [t01[:, 0:CH], t01[:, bass.ds(CH, CH)],
                    t23[:, 0:CH], t23[:, bass.ds(CH, CH)]]
            accs += [ps512.tile([P, CH], DT, tag="ps512", name=f"op3c{j}")
                     for j in range(2)]
            # k=0 as closed groups so the scheduler runs them during the den
            # chain instead of pairing each with its (blocked) k=1 partner;
            # k=1 then accumulates onto the surviving has_written state
            for m in range(6):
                nc.tensor.matmul(
                    accs[m], lhsT=wobig[:, bass.ds(P * m, P)],
                    rhs=ohat_c[0][qi][:], start=True, stop=True)
            for m in range(6):
                nc.tensor.matmul(
                    accs[m], lhsT=wobig[:, bass.ds(EMBED + P * m, P)],
                    rhs=ohat_c[1][qi][:], start=False, stop=True,
                    skip_group_check=True)
                cast_and_dma(m, accs[m], 2)
            for m in (6, 7):
                ps = stP.tile([P, CH], DT, tag="stP", name="pso")
                for k in range(2):
                    nc.tensor.matmul(
                        ps,
                        lhsT=wobig[:, bass.ds(EMBED * k + P * m, P)],
                        rhs=ohat_c[k][qi][:],
                        start=(k == 0), stop=(k == 1),
                    )
                cast_and_dma(m, ps, 2)

        # software pipeline: fill PE stalls at pair boundaries with
        # independent QKV / out-proj work
        # schedule: every attn pair is exp-bound (ACT > its own PE work), so
        # the pure-PE fill work (qkv, outproj) is rationed across the windows:
        # qkv(qi+1) fills the qi window, and ALL outproj chunks are held back
        # for the heavily exp-bound qi=3 window.
        qkv_chunk(0, x0)
        attn_pair(0, 0)
        qkv_chunk(1, x1)
        attn_pair(1, 0)
        attn_pair(0, 1)
        qkv_chunk(2, x2)
        attn_pair(1, 1)
        attn_pair(0, 2)
        qkv_chunk(3, x3)
        attn_pair(1, 2)
        attn_pair(0, 3)
        outproj_chunk(0)
        attn_pair(1, 3)
        outproj_chunk(1)
        outproj_chunk(2)
        outproj_chunk(NCH - 1, pool=stP)

    nc.compile()
    return nc


def kernel(x, W_qkv, b_qkv, W_out, b_out):
    global LAST_EXEC_NS, LAST_RESULTS
    x = np.asarray(x, dtype=np.float32)
    W_qkv = np.asarray(W_qkv, dtype=np.float32)
    b_qkv = np.asarray(b_qkv, dtype=np.float32)
    W_out = np.asarray(W_out, dtype=np.float32)
    b_out = np.asarray(b_out, dtype=np.float32)

    nc = _build_program()

    tri = (np.arange(P)[:, None] <= np.arange(P)[None, :]).astype(np.float32)
    mask2 = np.concatenate([tri, tri], axis=1)          # [128, 256]

    in_maps = []
    for c in range(N_CORES):
        b, g = divmod(c, HPC)
        q0 = GCOLS * g
        wq = W_qkv[q0:q0 + GCOLS]                        # [256, 1024]
        wk = W_qkv[EMBED + q0:EMBED + q0 + GCOLS]
        wv = W_qkv[2 * EMBED + q0:2 * EMBED + q0 + GCOLS]
        bq = b_qkv[q0:q0 + GCOLS]
        bk = b_qkv[EMBED + q0:EMBED + q0 + GCOLS]
        bqk_h = np.stack([bq[0:P], bq[P:2 * P], bk[0:P], bk[P:2 * P]],
                         axis=1).astype(np.float32)      # [128, 4]
        xT = np.ascontiguousarray(x[b].T)                # [1024, 2048]
        xtile = xT.reshape(8, P, NCH, CH).transpose(1, 2, 0, 3).reshape(P, -1)
        wqkT = np.concatenate([wq, wk], 0).T             # [1024, 512]
        wqkt = wqkT.reshape(8, P, 2 * GCOLS).transpose(1, 0, 2).reshape(P, -1)
        wvt = wv.T.reshape(8, P, GCOLS).transpose(1, 0, 2).reshape(P, -1)
        wot = W_out[:, q0:q0 + GCOLS].T.reshape(2, P, EMBED).transpose(1, 0, 2).reshape(P, -1)
        in_maps.append({
            "xT": np.ascontiguousarray(xtile).astype(ml_dtypes.bfloat16),
            "wqkT": np.ascontiguousarray(wqkt).astype(ml_dtypes.bfloat16),
            "wvT": np.ascontiguousarray(wvt).astype(ml_dtypes.bfloat16),
            "bqk": np.ascontiguousarray(bqk_h),
            "woT": np.ascontiguousarray(wot).astype(ml_dtypes.bfloat16),
            "mask2": mask2.astype(ml_dtypes.bfloat16),
        })

    want_trace = bool(int(os.environ.get("KTRACE", "0")))
    if want_trace:
        try:
            import antenv.axon_hooks  # noqa: F401
        except ImportError:
            want_trace = False
    res = run_bass_kernel_spmd(nc, in_maps, list(range(N_CORES)),
                               trace=want_trace,
                               tmpdir=os.environ.get("KTRACE_DIR") or None)
    LAST_EXEC_NS = res.exec_time_ns
    LAST_RESULTS = res

    out = np.empty((BATCH, SEQ, EMBED), dtype=np.float32)
    crow = (b_out + W_out @ b_qkv[2 * EMBED:]).astype(np.float32)
    for b in range(BATCH):
        acc = np.zeros((EMBED, SEQ), dtype=np.float32)
        for g in range(HPC):
            yt = res.results[HPC * b + g]["yT"].astype(np.float32)
            acc += yt.reshape(P, NCH, 8, CH).transpose(2, 0, 1, 3).reshape(EMBED, SEQ)
        out[b] = acc.T + crow[None, :]
    return out


# revision 51
# speedup vs baseline: 1.1899x; 1.0237x over previous
"""Multi-head self-attention (batch=2, seq=2048, embed=1024, heads=16, causal)
sharded over 8 NeuronCores: data-parallel over batch (x2) and tensor-parallel
over heads (x4 groups of 4 heads).

Each core computes, for its (batch b, head group g):
  qkvT proj (transposed activations), causal softmax attention with the
  denominator folded into the AV matmul via a ones-column on V, and a partial
  output projection W_out[:, cols_g].T @ o_hat in transposed layout.
Host sums the 4 partials per batch, transposes back, and adds the constant
row  b_out + W_out @ b_v  (exact bias algebra).

Perf structure: all inputs are pre-tiled host-side so every DMA is a single
contiguous [128, N] transfer; diagonal k-tiles are trimmed to the causal
region (matmul N, exp N, and the mask mul all shrink); the K=64 logits
matmuls row-tile two heads concurrently on the PE array (auto tile_position
from base_partition).
"""

import os

import ml_dtypes
import numpy as np
from contextlib import ExitStack

import concourse.bass as bass
import concourse.mybir as mybir
import concourse.tile as tile
from concourse import bacc
from concourse.bass_utils import run_bass_kernel_spmd

N_HEADS = 16
EMBED = 1024
HEAD = 64
SEQ = 2048
BATCH = 2
N_CORES = 8
HPC = 4                # heads per core
GCOLS = HPC * HEAD     # 256 embed columns per head group
P = 128
CH = 512               # seq chunk
NCH = SEQ // CH        # 4
KT = SEQ // P          # 16 k tiles
VW = HPC * (HEAD + 1)  # v tile width per ktile (ones column at HEAD)

DT = mybir.dt.float32
DTB = mybir.dt.bfloat16
DTH = mybir.dt.float16

LAST_EXEC_NS = None
LAST_RESULTS = None


def _build_program():
    nc = bacc.Bacc("TRN2", target_bir_lowering=False, debug=False,
                   num_devices=N_CORES)
    # pre-tiled host layouts: every tensor is [128, ...] with the exact SBUF
    # column order so each load is one contiguous DMA
    xT = nc.dram_tensor("xT", [P, NCH * 8 * CH], DTB, kind="ExternalInput")
    wqkT = nc.dram_tensor("wqkT", [P, 8 * 2 * GCOLS], DTB, kind="ExternalInput")
    wvT = nc.dram_tensor("wvT", [P, 8 * GCOLS], DTB, kind="ExternalInput")
    bqk = nc.dram_tensor("bqk", [P, 4], DT, kind="ExternalInput")
    woT = nc.dram_tensor("woT", [P, 2 * EMBED], DTB, kind="ExternalInput")
    mask2 = nc.dram_tensor("mask2", [P, 2 * P], DTB, kind="ExternalInput")
    yT = nc.dram_tensor("yT", [P, NCH * 8 * CH], DTH, kind="ExternalOutput")

    with tile.TileContext(nc) as tc, ExitStack() as ctx:
        const = ctx.enter_context(tc.tile_pool(name="const", bufs=1))
        xpool = ctx.enter_context(tc.tile_pool(name="xpool", bufs=4))
        stpool = ctx.enter_context(tc.tile_pool(name="stpool", bufs=10))
        small = ctx.enter_context(tc.tile_pool(name="small", bufs=6))
        outsb = ctx.enter_context(tc.tile_pool(name="outsb", bufs=3))
        ps512 = ctx.enter_context(tc.tile_pool(name="ps512", bufs=2, space="PSUM"))
        stP = ctx.enter_context(tc.tile_pool(name="stP", bufs=2, space="PSUM"))
        psO = ctx.enter_context(tc.tile_pool(name="psO", bufs=2, space="PSUM"))

        # ---- persistent SBUF residents ----
        wqkbig = const.tile([P, 8 * 2 * GCOLS], DTB, tag="wqkbig")
        wvbig = const.tile([P, 8 * GCOLS], DTB, tag="wvbig")
        wobig = const.tile([P, 2 * EMBED], DTB, tag="wobig")
        bqk_sb = const.tile([P, 4], DT, tag="bqk")
        mask_sb = const.tile([P, 2 * P], DTB, tag="mask")
        # per-chunk tiles: the Tile framework tracks deps at tile granularity,
        # so chunked q/k/v/ohat avoid false write-read serialization between
        # qkv of a later chunk and attention/outproj of an earlier one
        qt_c = [[const.tile([P, CH], DTB, tag=f"qt{a}c{q}", name=f"qt{a}c{q}")
                 for q in range(NCH)] for a in range(2)]
        kt_c = [[const.tile([P, CH], DTB, tag=f"kt{a}c{q}", name=f"kt{a}c{q}")
                 for q in range(NCH)] for a in range(2)]
        vt_c = [const.tile([P, 4 * VW], DTB, tag=f"vtc{q}", name=f"vtc{q}")
                for q in range(NCH)]
        ohat_c = [[const.tile([P, CH], DTB, tag=f"oh{a}c{q}", name=f"oh{a}c{q}")
                   for q in range(NCH)] for a in range(2)]

        def load_x(qi, eng):
            xb = xpool.tile([P, 8 * CH], DTB, tag="xt", name=f"x{qi}")
            eng.dma_start(out=xb, in_=xT[:, bass.ds(8 * CH * qi, 8 * CH)])
            return xb

        # initial loads. wqk and x0 are pieced (256KB) and alternated across
        # the two HWDGE rings (SDMA round-robins rings at packet granularity,
        # so both rings must carry critical bytes); everything non-critical
        # queues strictly behind them to keep full HBM bandwidth on the
        # chunk-0 critical path.
        x0 = xpool.tile([P, 8 * CH], DTB, tag="xt", name="x0")
        # piece widths: a tiny first piece (i=0 only) lets the first matmul
        # start ~2us earlier; later pieces are bigger to amortize issue cost
        pieces = [(0, CH), (CH, 3 * CH), (4 * CH, 2 * CH), (6 * CH, 2 * CH)]
        for pc, (o, w) in enumerate(pieces):
            sl = bass.ds(o, w)
            ew, ex = (nc.sync, nc.scalar) if pc % 2 == 0 else (nc.scalar, nc.sync)
            ew.dma_start(out=wqkbig[:, sl], in_=wqkT[:, sl])
            ex.dma_start(out=x0[:, sl], in_=xT[:, sl])
        nc.gpsimd.dma_start(out=bqk_sb, in_=bqk[:])
        nc.gpsimd.dma_start(out=mask_sb, in_=mask2[:])
        nc.sync.dma_start(out=wvbig, in_=wvT[:])
        x1 = load_x(1, nc.sync)
        x2 = load_x(2, nc.scalar)
        x3 = load_x(3, nc.sync)
        nc.scalar.dma_start(out=wobig, in_=woT[:])
        # ones column per (ktile, head) at offset HEAD in each v block
        for q in range(NCH):
            nc.vector.memset(
                vt_c[q].rearrange("p (t h d) -> p t h d",
                                  t=4, h=HPC)[:, :, :, HEAD:HEAD + 1],
                1.0)


        def qkv_chunk(qi, xb):
            sl = bass.ds(CH * qi, CH)
            if qi == 0:
                # i-major with 4 accumulators (attn pools are idle during
                # chunk 0, so f2/f3 borrow stP) — all 4 f matmuls consume each
                # x/wqk piece as it lands, keeping PE at DMA arrival rate
                pss = [ps512.tile([P, CH], DT, tag="ps512", name=f"psqk{f}")
                       for f in range(2)]
                pss += [stP.tile([P, CH], DT, tag="stP", name=f"psqk{f}")
                        for f in range(2, 4)]
                for i in range(8):
                    for f in range(4):
                        nc.tensor.matmul(
                            pss[f],
                            lhsT=wqkbig[:, bass.ds(2 * GCOLS * i + P * f, P)],
                            rhs=xb[:, bass.ds(CH * i, CH)],
                            start=(i == 0), stop=(i == 7),
                        )
                for f in range(4):
                    dst = qt_c[f][qi] if f < 2 else kt_c[f - 2][qi]
                    # scalar is exp-free this early: K casts run there so the
                    # Q/K casts pair up and a00's logits start ~1us sooner
                    if f >= 2:
                        nc.scalar.add(dst[:], pss[f], bqk_sb[:, f:f + 1])
                    else:
                        nc.vector.tensor_scalar_add(dst[:], pss[f],
                                                    bqk_sb[:, f:f + 1])
            else:
                for f in range(4):
                    ps = ps512.tile([P, CH], DT, tag="ps512", name="psqk")
                    for i in range(8):
                        nc.tensor.matmul(
                            ps,
                            lhsT=wqkbig[:, bass.ds(2 * GCOLS * i + P * f, P)],
                            rhs=xb[:, bass.ds(CH * i, CH)],
                            start=(i == 0), stop=(i == 7),
                        )
                    dst = qt_c[f][qi] if f < 2 else kt_c[f - 2][qi]
                    if qi == 1 and f >= 2:
                        nc.scalar.add(dst[:], ps, bqk_sb[:, f:f + 1])
                    else:
                        nc.vector.tensor_scalar_add(dst[:], ps,
                                                    bqk_sb[:, f:f + 1])
            for s in range(4):
                ps = ps512.tile([P, GCOLS], DT, tag="ps512", name="psv")
                for i in range(8):
                    nc.tensor.matmul(
                        ps,
                        lhsT=xb[:, bass.ds(CH * i + P * s, P)],
                        rhs=wvbig[:, bass.ds(GCOLS * i, GCOLS)],
                        start=(i == 0), stop=(i == 7),
                    )
                dst = vt_c[qi][:, bass.ds(VW * s, VW)].rearrange(
                    "p (h d) -> p h d", h=HPC)[:, :, 0:HEAD]
                nc.vector.tensor_copy(dst, ps.rearrange("p (h d) -> p h d", h=HPC))

        def attn_pair(hp, qi):
            nk = 4 * qi + 4
            po = [psO.tile([HEAD + 1, CH], DT, tag="psO", name="psO")
                  for _ in range(2)]
            for ki in range(nk):
                kr = ki - 4 * qi
                off = P * kr if kr > 0 else 0   # causal trim for diagonal tiles
                W = CH - off
                pst = stP.tile([P, 2 * CH], DT, tag="stP", name="pst")
                for hh in range(2):
                    r0 = HEAD * hh
                    nc.tensor.matmul(
                        pst[:, bass.ds(CH * hh + off, W)],
                        lhsT=kt_c[hp][ki // 4][r0:r0 + HEAD, bass.ds(P * (ki % 4), P)],
                        rhs=qt_c[hp][qi][r0:r0 + HEAD, bass.ds(off, W)],
                        start=True, stop=True,
                    )
                st = stpool.tile([P, 2 * CH], DTB, tag="st", name="st")
                stv = st[:, 0:2 * W].rearrange("p (h w) -> p h w", h=2)
                nc.scalar.activation(
                    stv,
                    pst.rearrange("p (h c) -> p h c", h=2)[:, :, off:CH],
                    mybir.ActivationFunctionType.Exp,
                    scale=0.125)
                if kr >= 0:
                    nc.vector.tensor_mul(
                        stv[:, :, 0:P], stv[:, :, 0:P],
                        mask_sb.rearrange("p (h c) -> p h c", h=2))
                for hh in range(2):
                    h = 2 * hp + hh
                    nc.tensor.matmul(
                        po[hh][:, bass.ds(off, W)],
                        lhsT=vt_c[ki // 4][:, bass.ds(VW * (ki % 4) + (HEAD + 1) * h,
                                                      HEAD + 1)],
                        rhs=st[:, bass.ds(W * hh, W)],
                        start=(ki == 0), stop=(ki == nk - 1),
                    )
            # per-hh pipelined den chain: copy/recip/broadcast/mul stages of
            # the two heads overlap across the vector and gpsimd queues
            last_pair = (hp == 1 and qi == NCH - 1)
            den = small.tile([1, 2 * CH], DT, tag="den", name="den")
            nc.vector.tensor_copy(den[:, 0:CH], po[0][HEAD:HEAD + 1, :])
            # scalar is exp-free at the very end: parallelize the tail chain
            (nc.scalar.copy if last_pair else nc.vector.tensor_copy)(
                den[:, bass.ds(CH, CH)], po[1][HEAD:HEAD + 1, :])
            recip = small.tile([1, 2 * CH], DT, tag="recip", name="recip")
            recipb = small.tile([HEAD, 2 * CH], DT, tag="recipb", name="recipb")
            for hh in range(2):
                nc.vector.reciprocal_approx_fast(
                    recip[:, bass.ds(CH * hh, CH)], den[:, bass.ds(CH * hh, CH)])
                nc.gpsimd.partition_broadcast(
                    recipb[:, bass.ds(CH * hh, CH)], recip[:, bass.ds(CH * hh, CH)])
            for hh in range(2):
                nc.vector.tensor_mul(
                    ohat_c[hp][qi][HEAD * hh:HEAD * hh + HEAD, :],
                    po[hh][0:HEAD, :], recipb[:, bass.ds(CH * hh, CH)])

        def outproj_chunk(qi, pool=None):
            last = pool is not None
            obig = outsb.tile([P, 8 * CH], DTH, tag="ot", name="obig")

            def cast_and_dma(m, ps, gsz):
                if last:
                    # scalar engine is exp-free by the last chunk
                    nc.scalar.copy(obig[:, bass.ds(CH * m, CH // 2)],
                                   ps[:, 0:CH // 2])
                    nc.vector.tensor_copy(
                        obig[:, bass.ds(CH * m + CH // 2, CH // 2)],
                        ps[:, bass.ds(CH // 2, CH // 2)])
                else:
                    nc.vector.tensor_copy(obig[:, bass.ds(CH * m, CH)], ps)
                if m % gsz == gsz - 1:
                    grp = m // gsz
                    deng = nc.scalar if (last and grp % 2 == 1) else nc.sync
                    deng.dma_start(
                        out=yT[:, bass.ds(8 * CH * qi + gsz * CH * grp, gsz * CH)],
                        in_=obig[:, bass.ds(gsz * CH * grp, gsz * CH)])

            if not last:
                for m in range(8):
                    ps = ps512.tile([P, CH], DT, tag="ps512", name="pso")
                    for k in range(2):
                        nc.tensor.matmul(
                            ps,
                            lhsT=wobig[:, bass.ds(EMBED * k + P * m, P)],
                            rhs=ohat_c[k][qi][:],
                            start=(k == 0), stop=(k == 1),
                        )
                    cast_and_dma(m, ps, 4)
                return
            # last chunk: two-phase — the k=0 matmuls only need the hp0 ohat
            # (ready long before) and run during the final den chain, keeping
            # the PE warm; k=1 lands as soon as the hp1 normalize finishes
            t01 = stP.tile([P, 2 * CH], DT, tag="stP", name="op3a")
            t23 = stP.tile([P, 2 * CH], DT, tag="stP", name="op3b")
            accs = [t01[:, 0:CH], t01[:, bass.ds(CH, CH)],
                    t23[:, 0:CH], t23[:, bass.ds(CH, CH)]]
            accs += [ps512.tile([P, CH], DT, tag="ps512", name=f"op3c{j}")
                     for j in range(2)]
            # k=0 as closed groups so the scheduler runs them during the den
            # chain instead of pairing each with its (blocked) k=1 partner;
            # k=1 then accumulates onto the surviving has_written state
            for m in range(6):
                nc.tensor.matmul(
                    accs[m], lhsT=wobig[:, bass.ds(P * m, P)],
                    rhs=ohat_c[0][qi][:], start=True, stop=True)
            for m in range(6):
                nc.tensor.matmul(
                    accs[m], lhsT=wobig[:, bass.ds(EMBED + P * m, P)],
                    rhs=ohat_c[1][qi][:], start=False, stop=True,
                    skip_group_check=True)
                cast_and_dma(m, accs[m], 2)
            for m in (6, 7):
                ps = stP.tile([P, CH], DT, tag="stP", name="pso")
                for k in range(2):
                    nc.tensor.matmul(
                        ps,
                        lhsT=wobig[:, bass.ds(EMBED * k + P * m, P)],
                        rhs=ohat_c[k][qi][:],
                        start=(k == 0), stop=(k == 1),
                    )
                cast_and_dma(m, ps, 2)

        # software pipeline: fill PE stalls at pair boundaries with
        # independent QKV / out-proj work
        # schedule: every attn pair is exp-bound (ACT > its own PE work), so
        # the pure-PE fill work (qkv, outproj) is rationed across the windows:
        # qkv(qi+1) fills the qi window, and ALL outproj chunks are held back
        # for the heavily exp-bound qi=3 window.
        qkv_chunk(0, x0)
        attn_pair(0, 0)
        qkv_chunk(1, x1)
        attn_pair(1, 0)
        attn_pair(0, 1)
        qkv_chunk(2, x2)
        attn_pair(1, 1)
        attn_pair(0, 2)
        qkv_chunk(3, x3)
        attn_pair(1, 2)
        attn_pair(0, 3)
        outproj_chunk(0)
        attn_pair(1, 3)
        outproj_chunk(1)
        outproj_chunk(2)
        outproj_chunk(NCH - 1, pool=stP)

    nc.compile()
    return nc


def kernel(x, W_qkv, b_qkv, W_out, b_out):
    global LAST_EXEC_NS, LAST_RESULTS
    x = np.asarray(x, dtype=np.float32)
    W_qkv = np.asarray(W_qkv, dtype=np.float32)
    b_qkv = np.asarray(b_qkv, dtype=np.float32)
    W_out = np.asarray(W_out, dtype=np.float32)
    b_out = np.asarray(b_out, dtype=np.float32)

    nc = _build_program()

    tri = (np.arange(P)[:, None] <= np.arange(P)[None, :]).astype(np.float32)
    mask2 = np.concatenate([tri, tri], axis=1)          # [128, 256]

    in_maps = []
    for c in range(N_CORES):
        b, g = divmod(c, HPC)
        q0 = GCOLS * g
        wq = W_qkv[q0:q0 + GCOLS]                        # [256, 1024]
        wk = W_qkv[EMBED + q0:EMBED + q0 + GCOLS]
        wv = W_qkv[2 * EMBED + q0:2 * EMBED + q0 + GCOLS]
        bq = b_qkv[q0:q0 + GCOLS]
        bk = b_qkv[EMBED + q0:EMBED + q0 + GCOLS]
        bqk_h = np.stack([bq[0:P], bq[P:2 * P], bk[0:P], bk[P:2 * P]],
                         axis=1).astype(np.float32)      # [128, 4]
        xT = np.ascontiguousarray(x[b].T)                # [1024, 2048]
        xtile = xT.reshape(8, P, NCH, CH).transpose(1, 2, 0, 3).reshape(P, -1)
        wqkT = np.concatenate([wq, wk], 0).T             # [1024, 512]
        wqkt = wqkT.reshape(8, P, 2 * GCOLS).transpose(1, 0, 2).reshape(P, -1)
        wvt = wv.T.reshape(8, P, GCOLS).transpose(1, 0, 2).reshape(P, -1)
        wot = W_out[:, q0:q0 + GCOLS].T.reshape(2, P, EMBED).transpose(1, 0, 2).reshape(P, -1)
        in_maps.append({
            "xT": np.ascontiguousarray(xtile).astype(ml_dtypes.bfloat16),
            "wqkT": np.ascontiguousarray(wqkt).astype(ml_dtypes.bfloat16),
            "wvT": np.ascontiguousarray(wvt).astype(ml_dtypes.bfloat16),
            "bqk": np.ascontiguousarray(bqk_h),
            "woT": np.ascontiguousarray(wot).astype(ml_dtypes.bfloat16),
            "mask2": mask2.astype(ml_dtypes.bfloat16),
        })

    want_trace = bool(int(os.environ.get("KTRACE", "0")))
    if want_trace:
        try:
            import antenv.axon_hooks  # noqa: F401
        except ImportError:
            want_trace = False
    res = run_bass_kernel_spmd(nc, in_maps, list(range(N_CORES)),
                               trace=want_trace,
                               tmpdir=os.environ.get("KTRACE_DIR") or None)
    LAST_EXEC_NS = res.exec_time_ns
    LAST_RESULTS = res

    out = np.empty((BATCH, SEQ, EMBED), dtype=np.float32)
    crow = (b_out + W_out @ b_qkv[2 * EMBED:]).astype(np.float32)
    for b in range(BATCH):
        acc = np.zeros((EMBED, SEQ), dtype=np.float32)
        for g in range(HPC):
            yt = res.results[HPC * b + g]["yT"].astype(np.float32)
            acc += yt.reshape(P, NCH, 8, CH).transpose(2, 0, 1, 3).reshape(EMBED, SEQ)
        out[b] = acc.T + crow[None, :]
    return out
